# revision 1
# baseline (speedup 1.0000x reference)
"""Trainium2 Bass kernel for nn_DAM_88519275970682.

Computes batched-prefix DAM InfoNCE loss + accuracy:
  loss, acc = reference(A_logits, B_logits, sequences, dataset, indices)

Strategy (8 NeuronCores, SPMD, prefix-length-parallel):
  - The 255 prefix rows r (=n-1) are interleaved across 8 cores
    (core c gets r = c, c+8, ...; 32 slots/core, core 7 pads one slot).
  - Per core and per r (device, all fp32):
      E  = exp(A_logits[r+1].T)  with rows i>r masked to -1e4 (host-baked)
      hatT[h,b] = (E.T@zeta)[h,b] / Z[h]   (Z via an appended ones column)
      logits[b,:] = hatT.T @ phi_allT      (PE fp32, K=2048 in 4 chunks)
      per 512-chunk: neg rowmax + sum(exp(x-rowmax))    [flash-style]
      val[b] = diag(hatT.T @ phi_selT) == logits[b, idx_b] bit-exactly
  - phi_allT = W@dataset.T, phi_selT = W@sequences.T (W=softmax(B_logits));
    sequences = dataset[indices], so phi_selT columns are bit-identical to
    gathered phi_allT columns and match=(val==rowmax) reproduces argmax
    equality exactly.
  - Host combines per-(r,b) pieces in float64: M=-max(nm),
    ce = M + log(sum_c s_c*exp(m_c-M)) - val; loss/acc are means.
"""
import numpy as np
from contextlib import ExitStack

N, H, K, B = 256, 512, 2048, 256
NCORES = 8
NR = 32          # r-slots per core (core 7: last slot is padding)
NR1 = 16         # first NR1 slots have r = c + 8j <= 127 on every core
MASK_VAL = np.float32(-1e4)   # exp(-1e4) == 0.0 in fp32

# blob column layout (per-partition f32 offsets)
BL_BL = 0              # B_logits as 4 h-tiles of (128, 256)
BL_ZT = 1024           # zetaT_ext: 2 i-tiles of (128, 257)
BL_EYE = BL_ZT + 514   # eye(128)
BLW = BL_EYE + 128


def build_program(nr=NR, bufs_cfg=None):
    import concourse.bacc as bacc
    import concourse.mybir as mybir
    import concourse.tile as tile

    F32 = mybir.dt.float32
    AF = mybir.ActivationFunctionType
    ALU = mybir.AluOpType
    AX = mybir.AxisListType

    nc = bacc.Bacc("TRN2", target_bir_lowering=False, debug=False)

    nr1 = min(nr, NR1)   # slots with r < 128: only i-tile 0 needed
    nr2 = nr - nr1
    blob_in = nc.declare_dram_parameter("blob_in", [128, BLW], F32, isOutput=False)
    dst_in = nc.declare_dram_parameter("dst_in", [2, 128, K], F32, isOutput=False)
    msk_in = nc.declare_dram_parameter("msk_in", [128, 2 * K], F32, isOutput=False)
    a1_in = nc.declare_dram_parameter("a1_in", [max(nr1, 1), 128, 512], F32, isOutput=False)
    a2_in = nc.declare_dram_parameter("a2_in", [max(nr2, 1), 256, 512], F32, isOutput=False)
    resA_out = nc.declare_dram_parameter("resA_out", [128, NR * 2 * 5], F32, isOutput=True)
    resB_out = nc.declare_dram_parameter("resB_out", [128, NR * 2 * 4], F32, isOutput=True)

    with tile.TileContext(nc) as tc, ExitStack() as ctx:
        sb = ctx.enter_context(tc.tile_pool(name="sb", bufs=1))

        BF16 = mybir.dt.bfloat16
        blob = sb.tile([128, BLW], F32, tag="blob")
        # phi_allT split into bf16 hi+lo (q-region q*2048 + kc*512)
        pa_hi = sb.tile([128, 4 * 2048], BF16, tag="pa_hi")
        pa_lo = sb.tile([128, 4 * 2048], BF16, tag="pa_lo")
        msk = sb.tile([128, 2 * K], F32, tag="msk")       # one-hot idx masks per blk
        resA = sb.tile([128, NR * 2 * 5], F32, tag="resA")  # [nm0..3, val] per (j,blk)
        resB = sb.tile([128, NR * 2 * 4], F32, tag="resB")  # [s0..3] per (j,blk)

        nc.gpsimd.dma_start(blob[:], blob_in[:])
        nc.gpsimd.dma_start(msk[:], msk_in[:])

        # zeta is +-1 (and the ones column), exactly representable in bf16
        ztb = sb.tile([128, 514], BF16, tag="ztb")
        nc.vector.tensor_copy(ztb[:], blob[:, BL_ZT:BL_ZT + 514])

        def zt(t):
            return ztb[:, t * 257: (t + 1) * 257]

        eyeb = blob[:, BL_EYE:BL_EYE + 128]

        # ---------------- setup ----------------
        with ExitStack() as sctx:
            ssb = sctx.enter_context(tc.tile_pool(name="ssb", bufs=1))
            sps = sctx.enter_context(tc.tile_pool(name="sps", bufs=2, space="PSUM"))

            dstr = ssb.tile([128, 2 * K], F32, tag="dstr")
            nc.gpsimd.dma_start(dstr.rearrange("p (t f) -> p t f", t=2),
                                dst_in.rearrange("t p f -> p t f"))

            # W = softmax(B_logits) along the free (n) axis
            ew = ssb.tile([128, 1024], F32, tag="ew")
            zw = ssb.tile([128, 4], F32, tag="zw")
            for q in range(4):
                nc.scalar.activation(ew[:, q * 256:(q + 1) * 256],
                                     blob[:, q * 256:(q + 1) * 256],
                                     AF.Exp, accum_out=zw[:, q:q + 1])
            rzw = ssb.tile([128, 4], F32, tag="rzw")
            nc.vector.reciprocal(rzw[:], zw[:])
            wsm = ssb.tile([128, 1024], F32, tag="wsm")
            for q in range(4):
                nc.vector.tensor_scalar_mul(wsm[:, q * 256:(q + 1) * 256],
                                            ew[:, q * 256:(q + 1) * 256],
                                            rzw[:, q:q + 1])

            # wt[n-tile t, h] = W.T, split to bf16 hi+lo (dataset is +-1,
            # bf16-exact, so 2 split terms give fp32-grade phi)
            wt_hi = ssb.tile([128, 1024], BF16, tag="wt_hi")
            wt_lo = ssb.tile([128, 1024], BF16, tag="wt_lo")
            for q in range(4):
                for t in range(2):
                    tp = sps.tile([128, 128], F32, tag="tp", name=f"tp{q}{t}")
                    nc.tensor.transpose(
                        tp[:], wsm[:, q * 256 + t * 128: q * 256 + t * 128 + 128],
                        eyeb)
                    wsl = slice(t * 512 + q * 128, t * 512 + q * 128 + 128)
                    nc.vector.tensor_copy(wt_hi[:, wsl], tp[:])
                    nc.vector.tensor_tensor(out=wt_lo[:, wsl], in0=tp[:],
                                            in1=wt_hi[:, wsl], op=ALU.subtract)

            dstb = ssb.tile([128, 2 * K], BF16, tag="dstb")
            nc.vector.tensor_copy(dstb[:], dstr[:])   # +-1, exact in bf16

            # phi_allT (pa), split into bf16 hi + lo: x = hi + lo captures
            # 16 mantissa bits; the logits matmul runs 3 bf16 terms
            # (hi*hi + hi*lo + lo*hi) at 1 cyc/row vs fp32's 4 cyc/row.
            for q in range(4):
                for kc in range(4):
                    pp = sps.tile([128, 512], F32, tag="pp", name=f"pp{q}{kc}")
                    for t in range(2):
                        wsl = slice(t * 512 + q * 128, t * 512 + q * 128 + 128)
                        dsl = slice(t * K + kc * 512, t * K + (kc + 1) * 512)
                        for wi, wpart in enumerate((wt_hi, wt_lo)):
                            nc.tensor.matmul(
                                pp[:], wpart[:, wsl], dstb[:, dsl],
                                start=(t == 0 and wi == 0),
                                stop=(t == 1 and wi == 1))
                    sl = slice(q * 2048 + kc * 512, q * 2048 + (kc + 1) * 512)
                    nc.vector.tensor_copy(pa_hi[:, sl], pp[:])
                    nc.vector.tensor_tensor(out=pa_lo[:, sl], in0=pp[:],
                                            in1=pa_hi[:, sl], op=ALU.subtract)

        # ---------------- main loop ----------------
        bc = bufs_cfg or {}
        aip = ctx.enter_context(tc.tile_pool(name="aip", bufs=bc.get("aip", 3)))
        ehp = ctx.enter_context(tc.tile_pool(name="ehp", bufs=bc.get("eh", 2)))
        hatp = ctx.enter_context(tc.tile_pool(name="hatp", bufs=bc.get("hat", 2)))
        rzp = ctx.enter_context(tc.tile_pool(name="rzp", bufs=bc.get("rz", 2)))
        scrp = ctx.enter_context(tc.tile_pool(name="scrp", bufs=bc.get("scr", 2)))
        vscrp = ctx.enter_context(tc.tile_pool(name="vscrp", bufs=bc.get("vscr", 2)))
        v4p = ctx.enter_context(tc.tile_pool(name="v4p", bufs=bc.get("v4", 2)))
        hp = ctx.enter_context(tc.tile_pool(name="hp", bufs=bc.get("hp", 3), space="PSUM"))
        lg = ctx.enter_context(tc.tile_pool(name="lg", bufs=bc.get("lg", 5), space="PSUM"))

        for j in range(nr):
            # slots j < nr1 hold r = c + 8j < 128: rows i >= 128 of E are
            # exactly zero, so the second i-tile contributes nothing and
            # is skipped entirely (half the hat matmuls + half the DMA).
            two = j >= nr1
            if two:
                ai = aip.tile([128, 1024], F32, tag="ai2", name=f"ai2_{j}")
                nc.sync.dma_start(ai.rearrange("p (t f) -> p t f", t=2),
                                  a2_in[j - nr1].rearrange("(t p) f -> p t f", p=128))
            else:
                ai = aip.tile([128, 512], F32, tag="ai1", name=f"ai1_{j}")
                nc.sync.dma_start(ai[:], a1_in[j])
            nc.scalar.activation(ai[:], ai[:], AF.Exp)   # E = exp(a), in place
            # E split to bf16 hi+lo (zeta is bf16-exact, 2 terms suffice)
            aw = 1024 if two else 512
            e_hi = ehp.tile([128, aw], BF16, tag="e_hi2" if two else "e_hi1",
                            name=f"e_hi{j}")
            e_lo = ehp.tile([128, aw], BF16, tag="e_lo2" if two else "e_lo1",
                            name=f"e_lo{j}")
            nc.scalar.copy(e_hi[:], ai[:])
            nc.vector.tensor_tensor(out=e_lo[:], in0=ai[:], in1=e_hi[:],
                                    op=ALU.subtract)

            hat_hi = hatp.tile([128, 1024], BF16, tag="hat_hi")
            hat_lo = hatp.tile([128, 1024], BF16, tag="hat_lo")
            rz = rzp.tile([128, 4], F32, tag="rz")
            for q in range(4):
                hps = hp.tile([128, 257], F32, tag="hp", name=f"hps{j}_{q}")
                nt = 2 if two else 1
                for t in range(nt):
                    esl = slice(t * 512 + q * 128, t * 512 + q * 128 + 128)
                    for ei, epart in enumerate((e_hi, e_lo)):
                        nc.tensor.matmul(
                            hps[:], epart[:, esl], zt(t),
                            start=(t == 0 and ei == 0),
                            stop=(t == nt - 1 and ei == 1))
                nc.vector.reciprocal(rz[:, q:q + 1], hps[:, 256:257])
                qs = slice(q * 256, (q + 1) * 256)
                # hi = round_bf16(U/Z); lo = round_bf16(U/Z - hi); the
                # mult is recomputed identically so hi+lo is a true split
                nc.vector.tensor_scalar_mul(hat_hi[:, qs],
                                            hps[:, 0:256], rz[:, q:q + 1])
                nc.vector.scalar_tensor_tensor(
                    out=hat_lo[:, qs], in0=hps[:, 0:256],
                    scalar=rz[:, q:q + 1], in1=hat_hi[:, qs],
                    op0=ALU.mult, op1=ALU.subtract)

            for blk in range(2):
                base5 = (j * 2 + blk) * 5
                base4 = (j * 2 + blk) * 4
                v4 = v4p.tile([128, 4], F32, tag="v4")
                for kc in range(4):
                    lgp = lg.tile([128, 512], F32, tag="lg", name=f"lg{j}_{blk}_{kc}")
                    for q in range(4):
                        hsl = slice(q * 256 + blk * 128, q * 256 + blk * 128 + 128)
                        psl = slice(q * 2048 + kc * 512, q * 2048 + (kc + 1) * 512)
                        for ti, (lh, rh) in enumerate(
                                ((hat_hi, pa_hi), (hat_hi, pa_lo), (hat_lo, pa_hi))):
                            nc.tensor.matmul(
                                lgp[:], lh[:, hsl], rh[:, psl],
                                start=(q == 0 and ti == 0),
                                stop=(q == 3 and ti == 2))
                    nc.vector.tensor_reduce(
                        out=resA[:, base5 + kc: base5 + kc + 1], in_=lgp[:],
                        axis=AX.X, op=ALU.max, negate=True)
                    # val gather: one-hot mask picks logits[b, idx_b]
                    # bit-exactly out of the live chunk (one nonzero/row)
                    vscr = vscrp.tile([128, 512], F32, tag="vscr")
                    nc.vector.tensor_tensor(
                        out=vscr[:], in0=lgp[:],
                        in1=msk[:, blk * K + kc * 512: blk * K + (kc + 1) * 512],
                        op=ALU.mult)
                    nc.vector.tensor_reduce(
                        out=v4[:, kc: kc + 1], in_=vscr[:],
                        axis=AX.X, op=ALU.add)
                    scr = scrp.tile([128, 512], F32, tag="scr")
                    nc.scalar.activation(
                        scr[:], lgp[:], AF.Exp,
                        bias=resA[:, base5 + kc: base5 + kc + 1],
                        accum_out=resB[:, base4 + kc: base4 + kc + 1])
                nc.vector.tensor_reduce(
                    out=resA[:, base5 + 4: base5 + 5], in_=v4[:],
                    axis=AX.X, op=ALU.add)

        nc.gpsimd.dma_start(resA_out[:], resA[:])
        nc.gpsimd.dma_start(resB_out[:], resB[:])

    nc.compile()
    return nc


def _prep_inputs(A_logits, B_logits, sequences, dataset, indices):
    """Host-side slicing/layout. Returns per-core input maps."""
    A_logits = np.ascontiguousarray(A_logits, dtype=np.float32)
    B_logits = np.ascontiguousarray(B_logits, dtype=np.float32)
    sequences = np.ascontiguousarray(sequences, dtype=np.float32)
    dataset = np.ascontiguousarray(dataset, dtype=np.float32)
    idx = np.asarray(indices).astype(np.int64)

    # one-hot gather masks: msk[p, blk*K + k] = 1 iff indices[blk*128+p]==k
    msk = np.zeros((128, 2 * K), np.float32)
    for blk in range(2):
        msk[np.arange(128), blk * K + idx[blk * 128: blk * 128 + 128]] = 1.0

    blob = np.zeros((128, BLW), np.float32)
    blob[:, BL_BL:BL_BL + 1024] = (
        B_logits.reshape(4, 128, 256).transpose(1, 0, 2).reshape(128, 1024))
    ztx = np.concatenate([sequences.T, np.ones((N, 1), np.float32)], axis=1)
    blob[:, BL_ZT:BL_ZT + 514] = (
        ztx.reshape(2, 128, 257).transpose(1, 0, 2).reshape(128, 514))
    blob[:, BL_EYE:BL_EYE + 128] = np.eye(128, dtype=np.float32)

    dst = np.ascontiguousarray(dataset.T.reshape(2, 128, K))

    # (n-1, i, h) layout once; per-core slices + causal masking after
    AP = np.ascontiguousarray(A_logits[1:].transpose(0, 2, 1))

    in_maps = []
    r_lists = []
    ii = np.arange(256)[None, :, None]
    for c in range(NCORES):
        rs = np.arange(c, N - 1, NCORES)
        r_lists.append(list(rs))
        rs1, rs2 = rs[:NR1], rs[NR1:]
        a1 = AP[rs1, 0:128, :].copy()
        np.copyto(a1, MASK_VAL, where=(ii[:, 0:128] > rs1[:, None, None]))
        a2 = np.empty((NR - NR1, 256, 512), np.float32)
        a2[:len(rs2)] = AP[rs2]
        np.copyto(a2[:len(rs2)], MASK_VAL,
                  where=(ii > rs2[:, None, None]))
        if len(rs2) < NR - NR1:
            a2[len(rs2):] = 0.0  # padding slots: exp(0)=1, discarded
        in_maps.append({"blob_in": blob, "dst_in": dst, "msk_in": msk,
                        "a1_in": a1, "a2_in": a2})
    return in_maps, r_lists


def _combine(results, r_lists):
    """Host float64 reduction of per-core [nm,s,val] pieces."""
    tot_ce = 0.0
    tot_match = 0
    cnt = 0
    for c in range(NCORES):
        resA = results[c]["resA_out"].astype(np.float64)
        resB = results[c]["resB_out"].astype(np.float64)
        for s, r in enumerate(r_lists[c]):
            for blk in range(2):
                a = resA[:, (s * 2 + blk) * 5:(s * 2 + blk) * 5 + 5]
                sc = resB[:, (s * 2 + blk) * 4:(s * 2 + blk) * 4 + 4]
                nm = a[:, 0:4]
                val = a[:, 4]
                m = -nm
                M = m.max(axis=1)
                S = (sc * np.exp(m - M[:, None])).sum(axis=1)
                ce = M + np.log(S) - val
                tot_ce += ce.sum()
                tot_match += int((val == M).sum())
                cnt += 128
    loss = np.float32(tot_ce / cnt)
    acc = np.float32(tot_match / cnt)
    return loss, acc


_CACHED_NC = None


def kernel(A_logits, B_logits, sequences, dataset, indices=None):
    from concourse.bass_utils import run_bass_kernel_spmd

    global _CACHED_NC
    if _CACHED_NC is None:
        _CACHED_NC = build_program()
    nc = _CACHED_NC

    in_maps, r_lists = _prep_inputs(A_logits, B_logits, sequences, dataset, indices)
    out = run_bass_kernel_spmd(nc, in_maps, list(range(NCORES)))
    loss, acc = _combine(out.results, r_lists)
    return loss, acc


def audit(nc=None):
    from collections import Counter
    if nc is None:
        nc = build_program()
    bad = 0
    cnt = Counter()
    for f in nc.m.functions:
        for bb in f.blocks:
            for inst in bb.instructions:
                si = inst.sync_info
                n = len(si.on_wait) if si else 0
                cnt[(type(inst).__name__, n)] += 1
                limit = 2 if type(inst).__name__ in ("InstEventSemaphore", "InstDrain") else 1
                if n > limit and type(inst).__name__ != "InstDrain":
                    bad += 1
                    if bad <= 25:
                        print("MULTIWAIT", inst.name, type(inst).__name__,
                              [(w.ant_name, w.wait_value) for w in si.on_wait])
    print(dict(cnt))
    print("bad:", bad)
    return bad


if __name__ == "__main__":
    import sys
    if len(sys.argv) > 1 and sys.argv[1] == "audit":
        audit()

